# revision 41
# baseline (speedup 1.0000x reference)
"""Trainium2 Bass kernel v4 for nn_CDE — feature-major, bf16, all-DVE einsum.

Structure (per core, Nc=136 lanes, 60 RK stages):
- Host precomputes: natural-spline derivative rows dx[s,n,d] (C60 @ knots)
  with the RK stage scale c_r folded in, pre-broadcast to 128 partitions in
  DRAM (bf16); the embed z0; weight transposes in bf16.
- Device stage: feature-major MLP (L0 4mm, L1/L2 16mm, L3 40mm, all bf16),
  tanh (bf16 out) with native per-partition bias on ACT.  The einsum is pure
  DVE: per d-pair a dense bf16 multiply against the DMA'd dx slice (2x mode,
  ~200ns) and an f32 running add; one fold -> kbf.  Post-tanh tail ~0.75us
  keeps the PE HAM clock warm (2.4 GHz).  RK updates (SBUF-only, ~1 stage of
  slack) run on gpsimd to keep DVE free.
"""
import os
import sys
import types

for _p in ("/opt/trn_rl_repo", "/root/.axon_site/_ro/trn_rl_repo"):
    if os.path.isdir(_p) and _p not in sys.path:
        sys.path.insert(0, _p)

if "antenv.axon_hooks" not in sys.modules:
    _m = types.ModuleType("antenv.axon_hooks")
    _hook = [None]

    def _set(hook):
        _hook[0] = hook

    def _get():
        if _hook[0] is None:
            try:
                from trn_agent_boot.trn_boot import _ntff_profile_via_ctypes
                _hook[0] = _ntff_profile_via_ctypes("/opt/axon/libaxon_pjrt.so")
            except Exception:
                pass
        return _hook[0]

    _m.set_axon_ntff_profile_hook = _set
    _m.get_axon_ntff_profile_hook = _get
    sys.modules["antenv.axon_hooks"] = _m

import numpy as np

N_CORES = 8
T, D, E, H = 16, 10, 128, 512
F3 = E * D
N_STEPS = T - 1
N_STAGES = 4 * N_STEPS  # 60
NC = 136               # lanes per core (8*136 = 1088 >= nact)
DNC = D * NC

last_results = None


def spline_stage_matrix(t):
    t = np.asarray(t, np.float64)
    Tn = len(t)
    h = np.diff(t)
    A = np.zeros((Tn, Tn))
    A[0, 0] = 1.0
    A[-1, -1] = 1.0
    for i in range(1, Tn - 1):
        A[i, i - 1] = h[i - 1]
        A[i, i] = 2.0 * (h[i - 1] + h[i])
        A[i, i + 1] = h[i]
    R = np.zeros((Tn, Tn))
    for i in range(1, Tn - 1):
        R[i, i - 1] = 6.0 / h[i - 1]
        R[i, i] = -6.0 / h[i - 1] - 6.0 / h[i]
        R[i, i + 1] = 6.0 / h[i]
    S = np.linalg.solve(A, R)
    Iden = np.eye(Tn)
    rows = []
    for j in range(Tn - 1):
        hs = h[j]
        for u_frac in (0.0, 1.0 / 3.0, 2.0 / 3.0, 1.0):
            s = t[j + 1] if u_frac == 1.0 else t[j] + u_frac * hs
            i = int(np.clip(np.searchsorted(t, s, side="right") - 1, 0, Tn - 2))
            u = s - t[i]
            b_row = (Iden[i + 1] - Iden[i]) / h[i] - h[i] * (2.0 * S[i] + S[i + 1]) / 6.0
            rows.append(b_row + u * S[i] + (u * u) / (2.0 * h[i]) * (S[i + 1] - S[i]))
    return np.asarray(rows), h


def w3_perm():
    fp = np.arange(F3)
    return (fp % E) * D + fp // E


def rk_scales(h):
    """c_r per stage so k'_r = c_r * k_r makes all RK updates plain adds."""
    c = np.empty(N_STAGES)
    for j in range(N_STEPS):
        hs = h[j]
        c[4 * j + 0] = hs / 3.0
        c[4 * j + 1] = hs
        c[4 * j + 2] = hs
        c[4 * j + 3] = hs / 8.0
    return c


def build_bass():
    import concourse.bass as bass  # noqa: F401
    import concourse.bacc as bacc
    import concourse.tile as tile
    import concourse.mybir as mybir

    F32 = mybir.dt.float32
    BF16 = mybir.dt.bfloat16
    AF = mybir.ActivationFunctionType
    ALU = mybir.AluOpType

    nc = bacc.Bacc("TRN2", target_bir_lowering=False)

    d_zin0 = nc.dram_tensor("zin0", [E, NC], BF16, kind="ExternalInput")
    d_z0 = nc.dram_tensor("z0", [E, NC], F32, kind="ExternalInput")
    d_dxall = nc.dram_tensor("dxall", [128, N_STAGES * DNC], BF16,
                             kind="ExternalInput")
    d_w0 = nc.dram_tensor("w0t", [E, H], BF16, kind="ExternalInput")
    d_w1 = nc.dram_tensor("w1t", [H, H], BF16, kind="ExternalInput")
    d_w2 = nc.dram_tensor("w2t", [H, H], BF16, kind="ExternalInput")
    d_w3 = nc.dram_tensor("w3pt", [H, F3], BF16, kind="ExternalInput")
    d_b012 = nc.dram_tensor("b012", [E, 12], F32, kind="ExternalInput")
    d_b3p = nc.dram_tensor("b3p", [E, D], F32, kind="ExternalInput")
    d_out = nc.dram_tensor("zout", [E, NC], F32, kind="ExternalOutput")

    with tile.TileContext(nc) as tc:
        with (
            tc.tile_pool(name="wpool", bufs=1) as wpool,
            tc.tile_pool(name="apool", bufs=2) as apool,
            tc.tile_pool(name="dpool", bufs=3) as dpool,
            tc.tile_pool(name="pmlp", bufs=3, space="PSUM") as pmlp,
            tc.tile_pool(name="p3p", bufs=4, space="PSUM") as p3p,
            tc.tile_pool(name="pjunk", bufs=1, space="PSUM") as pjunk,
        ):
            w0t = wpool.tile([E, H], BF16, tag="w0t")
            nc.sync.dma_start(out=w0t, in_=d_w0[:, :])
            w1k = [wpool.tile([128, H], BF16, tag=f"w1k{k}", name=f"w1k{k}")
                   for k in range(4)]
            w2k = [wpool.tile([128, H], BF16, tag=f"w2k{k}", name=f"w2k{k}")
                   for k in range(4)]
            w3k = [wpool.tile([128, F3], BF16, tag=f"w3k{k}", name=f"w3k{k}")
                   for k in range(4)]
            for k in range(4):
                nc.sync.dma_start(out=w1k[k], in_=d_w1[128 * k:128 * (k + 1), :])
                nc.sync.dma_start(out=w2k[k], in_=d_w2[128 * k:128 * (k + 1), :])
                nc.sync.dma_start(out=w3k[k], in_=d_w3[128 * k:128 * (k + 1), :])
            b012 = wpool.tile([E, 12], F32, tag="b012")
            nc.sync.dma_start(out=b012, in_=d_b012[:, :])
            b3p = wpool.tile([E, D], F32, tag="b3p")
            nc.sync.dma_start(out=b3p, in_=d_b3p[:, :])

            z0t = wpool.tile([E, NC], F32, tag="z0in")
            nc.sync.dma_start(out=z0t, in_=d_z0[:, :])
            zin0 = wpool.tile([E, NC], BF16, tag="zin0")
            nc.sync.dma_start(out=zin0, in_=d_zin0[:, :])

            junk = pjunk.tile([128, 512], F32, tag="junk")

            def fillers(n, rhs_ap):
                # keep-warm matmuls into a scratch PSUM bank: the PE HAM
                # clock gate re-throttles to 1.2 GHz if the PE sees idle
                # windows, so plug dependency-wait gaps with junk work.
                # rhs_ap gates WHEN they become runnable (the scheduler
                # reorders the PE stream, so data deps are the only
                # reliable placement).
                rhs_ap, fs = rhs_ap
                for i in range(n):
                    nc.tensor.matmul(junk[:, 0:fs], w0t[:, 0:128],
                                     rhs_ap, start=True, stop=True)

            dxb = {}

            def fetch_dx(s):
                if s >= N_STAGES:
                    return
                tl = dpool.tile([128, DNC], BF16, tag="dxb", name=f"dxb_{s}")
                nc.sync.dma_start(out=tl, in_=d_dxall[:, s * DNC:(s + 1) * DNC])
                dxb[s] = tl

            fetch_dx(0)
            fetch_dx(1)

            kp = [None] * 4
            z = z0t
            zjbf = zin0
            zb3bf = zb4bf = zprebf = zpre = s12 = None
            p0_pend = None  # psum pair tiles with base already accumulated
            kbf = None

            def relu(eng, out_ap, in_ap, bias_ap):
                if eng == "dve":
                    nc.vector.tensor_scalar(out=out_ap, in0=in_ap,
                                            scalar1=bias_ap, scalar2=0.0,
                                            op0=ALU.add, op1=ALU.max)
                else:
                    nc.scalar.activation(out_ap, in_ap, AF.Relu,
                                         bias=bias_ap, scale=1.0)

            def TT(out_ap, a_ap, b_ap, op=ALU.add):
                nc.vector.tensor_tensor(out=out_ap, in0=a_ap, in1=b_ap, op=op)

            def GT(out_ap, a_ap, b_ap, op=ALU.add):
                nc.vector.tensor_tensor(out=out_ap, in0=a_ap, in1=b_ap, op=op)

            R_ENG = ["dve", "act", "dve", "act"]

            for s in range(N_STAGES):
                j, r = divmod(s, 4)
                last = s == N_STAGES - 1

                # ---- L0 (psum may already hold base accumulation)
                if p0_pend is None:
                    p0 = [pmlp.tile([128, 2, 256], F32, tag="pmlp",
                                    name=f"p0a_{s}"),
                          pmlp.tile([128, 2, 256], F32, tag="pmlp",
                                    name=f"p0b_{s}")]
                    for m in range(4):
                        nc.tensor.matmul(p0[m >> 1][:, m & 1, 0:NC],
                                         w0t[:, 128 * m:128 * (m + 1)],
                                         zin0[:, :], start=True, stop=True)
                else:
                    p0 = p0_pend
                    for m in range(4):
                        nc.tensor.matmul(p0[m >> 1][:, m & 1, 0:NC],
                                         w0t[:, 128 * m:128 * (m + 1)],
                                         kbf[:, :],
                                         start=False, stop=((m & 1) == 1))
                fillers(4, (kbf[:, :], NC) if kbf is not None
                        else (zin0[:, :], NC))
                y0 = apool.tile([128, 4, NC], BF16, tag="y0", name=f"y0_{s}")
                for m in range(4):
                    relu(R_ENG[m], y0[:, m, :], p0[m >> 1][:, m & 1, 0:NC],
                         b012[:, m:m + 1])
                fetch_dx(s + 2)
                # ---- L1 (k-major so matmuls start after first relu chunk)
                p1 = [pmlp.tile([128, 2, 256], F32, tag="pmlp", name=f"p1a_{s}"),
                      pmlp.tile([128, 2, 256], F32, tag="pmlp", name=f"p1b_{s}")]
                for m in range(4):
                    for k in range(4):
                        nc.tensor.matmul(p1[m >> 1][:, m & 1, 0:NC],
                                         w1k[k][:, 128 * m:128 * (m + 1)],
                                         y0[:, k, :],
                                         start=((m & 1) == 0 and k == 0),
                                         stop=((m & 1) == 1 and k == 3))
                fillers(4, (y0[:, 0, :], NC))
                y1 = apool.tile([128, 4, NC], BF16, tag="y1", name=f"y1_{s}")
                for m in range(4):
                    relu(R_ENG[m], y1[:, m, :], p1[m >> 1][:, m & 1, 0:NC],
                         b012[:, 4 + m:5 + m])
                # ---- L2
                p2 = [pmlp.tile([128, 2, 256], F32, tag="pmlp", name=f"p2a_{s}"),
                      pmlp.tile([128, 2, 256], F32, tag="pmlp", name=f"p2b_{s}")]
                for m in range(4):
                    for k in range(4):
                        nc.tensor.matmul(p2[m >> 1][:, m & 1, 0:NC],
                                         w2k[k][:, 128 * m:128 * (m + 1)],
                                         y1[:, k, :],
                                         start=((m & 1) == 0 and k == 0),
                                         stop=((m & 1) == 1 and k == 3))
                fillers(4, (y1[:, 0, :], NC))
                y2 = apool.tile([128, 4, NC], BF16, tag="y2", name=f"y2_{s}")
                for m in range(4):
                    relu(R_ENG[m], y2[:, m, :], p2[m >> 1][:, m & 1, 0:NC],
                         b012[:, 8 + m:9 + m])

                # ---- L3 + tanh; bias is pre-added into PSUM by a K=2
                # one-hot matmul so each d-pair is ONE [128,272] tanh.  The
                # einsum is a per-pair dense bf16 multiply + bf16 running
                # add, so only pair 4's mult/add trail the last tanh.
                dxs = dxb.pop(s)
                sacc = None
                for p in range(5):
                    p3 = p3p.tile([128, 2, 256], F32, tag="p3", name=f"p3_{s}_{p}")
                    y3 = apool.tile([128, 2 * NC], BF16, tag="y3",
                                    name=f"y3_{s}_{p}")
                    for half in range(2):
                        dd = 2 * p + half
                        for k in range(4):
                            nc.tensor.matmul(p3[:, half, 0:NC],
                                             w3k[k][:, 128 * dd:128 * (dd + 1)],
                                             y2[:, k, :], start=(k == 0),
                                             stop=(k == 3))
                        nc.scalar.activation(y3[:, half * NC:(half + 1) * NC],
                                             p3[:, half, 0:NC], AF.Tanh,
                                             bias=b3p[:, dd:dd + 1], scale=1.0)
                    tmp = apool.tile([128, 2 * NC], BF16, tag="tmp",
                                     name=f"tmp_{s}_{p}")
                    TT(tmp, y3, dxs[:, 2 * p * NC:(2 * p + 2) * NC],
                       op=ALU.mult)
                    if p == 0:
                        sacc = tmp
                    else:
                        a = apool.tile([128, 2 * NC], BF16, tag="sacc",
                                       name=f"sacc_{s}_{p}")
                        TT(a, sacc, tmp)
                        sacc = a
                    if p == 3:
                        fillers(3, (y3[:, 0:NC], NC))
                    elif p == 4:
                        fillers(8, (y3[:, 0:NC], NC))

                # ---- off-path RK partials (gpsimd, SBUF-only, ~1 stage slack)
                if r == 0 and j > 0:
                    znew = apool.tile([E, NC], F32, tag="z", name=f"z_{j}")
                    GT(znew, zpre, kp[3])
                    z = znew
                    zjbf = apool.tile([E, NC], BF16, tag="zjbf", name=f"zjbf_{j}")
                    GT(zjbf, zpre, kp[3])
                elif r == 1:
                    zb3bf = apool.tile([E, NC], BF16, tag="zb3", name=f"zb3_{j}")
                    GT(zb3bf, z, kp[0], op=ALU.subtract)
                elif r == 2:
                    t4 = apool.tile([E, NC], F32, tag="t4", name=f"t4_{j}")
                    nc.vector.scalar_tensor_tensor(
                        out=t4, in0=kp[0], scalar=3.0, in1=z,
                        op0=ALU.mult, op1=ALU.add)
                    zb4bf = apool.tile([E, NC], BF16, tag="zb4", name=f"zb4_{j}")
                    GT(zb4bf, t4, kp[1], op=ALU.subtract)
                    s12 = apool.tile([E, NC], F32, tag="s12", name=f"s12_{j}")
                    GT(s12, kp[0], kp[1])
                elif r == 3:
                    s123 = apool.tile([E, NC], F32, tag="s123", name=f"s123_{j}")
                    GT(s123, s12, kp[2])
                    zpre = apool.tile([E, NC], F32, tag="zpre", name=f"zpre_{j}")
                    nc.vector.scalar_tensor_tensor(
                        out=zpre, in0=s123, scalar=0.375, in1=z,
                        op0=ALU.mult, op1=ALU.add)
                    zprebf = apool.tile([E, NC], BF16, tag="zprebf",
                                        name=f"zprebf_{j}")
                    nc.vector.tensor_copy(out=zprebf, in_=zpre)

                # ---- base L0 for next stage (runs during this stage's tail)
                if not last:
                    rn = (r + 1) % 4
                    base = (zprebf, zjbf, zb3bf, zb4bf)[rn]
                    p0_pend = [pmlp.tile([128, 2, 256], F32, tag="pmlp",
                                         name=f"p0a_{s + 1}"),
                               pmlp.tile([128, 2, 256], F32, tag="pmlp",
                                         name=f"p0b_{s + 1}")]
                    for m in range(4):
                        nc.tensor.matmul(p0_pend[m >> 1][:, m & 1, 0:NC],
                                         w0t[:, 128 * m:128 * (m + 1)],
                                         base[:, :],
                                         start=((m & 1) == 0), stop=False)

                # ---- finish k' (bf16, feeds both the L0 accumulation and
                # the RK updates)
                kbf = apool.tile([E, NC], BF16, tag="kbf", name=f"kbf_{s}")
                TT(kbf, sacc[:, 0:NC], sacc[:, NC:2 * NC])
                kp[r] = kbf

                if last:
                    zfin = apool.tile([E, NC], F32, tag="zfin", name="zfin")
                    TT(zfin, zpre, kbf)
                    z = zfin

            nc.sync.dma_start(out=d_out[:, :], in_=z)
    nc.finalize()
    return nc


_C60_H = None


def _prep_host(t, x, mask, W_embed, b_embed, W0, b0, W1, b1, W2, b2, W3, b3):
    import ml_dtypes
    bf16 = ml_dtypes.bfloat16

    t = np.asarray(t, np.float32)
    x = np.asarray(x, np.float32)
    mask = np.asarray(mask)
    B, Amax = mask.shape
    N = B * Amax

    C60, h = spline_stage_matrix(t)
    C60 = C60.astype(np.float32)
    idx = np.flatnonzero(mask.ravel())
    nact = len(idx)
    total = N_CORES * NC
    assert nact <= total, f"nact={nact} > {total}"
    pad = np.full(total, idx[0] if nact else 0, dtype=np.int64)
    pad[:nact] = idx
    xp = x.reshape(N, T, D)[pad]  # (total, T, D)

    c = rk_scales(h).astype(np.float32)
    perm = w3_perm()
    W3p = np.asarray(W3, np.float32)[perm]
    b3pv = np.asarray(b3, np.float32)[perm]

    shared = dict(
        w0t=np.ascontiguousarray(np.asarray(W0).T).astype(bf16),
        w1t=np.ascontiguousarray(np.asarray(W1).T).astype(bf16),
        w2t=np.ascontiguousarray(np.asarray(W2).T).astype(bf16),
        w3pt=np.ascontiguousarray(W3p.T).astype(bf16),
        b012=np.stack([np.asarray(b, np.float32)[m * 128:(m + 1) * 128]
                       for b in (b0, b1, b2) for m in range(4)],
                      axis=1).astype(np.float32),
        b3p=np.ascontiguousarray(b3pv.reshape(D, E).T).astype(np.float32),
    )

    Wemb = np.asarray(W_embed, np.float32)
    bemb = np.asarray(b_embed, np.float32)
    in_maps = []
    for core in range(N_CORES):
        xc = xp[core * NC:(core + 1) * NC]  # (NC, T, D)
        dx = np.einsum("st,ntd->snd", C60, xc)  # (60, NC, D)
        # (60, D, NC) row per stage, scaled by c; broadcast to 128 partitions
        dxc = (dx.transpose(0, 2, 1) * c[:, None, None]).reshape(1, N_STAGES * DNC)
        dxall = np.ascontiguousarray(
            np.broadcast_to(dxc, (128, N_STAGES * DNC))).astype(bf16)
        z0 = (xc[:, 0, :] @ Wemb.T + bemb).astype(np.float32).T  # (E, NC)
        in_maps.append(dict(
            zin0=np.ascontiguousarray(z0).astype(bf16),
            z0=np.ascontiguousarray(z0),
            dxall=dxall,
            **shared,
        ))
    return in_maps, pad, nact, h, C60, xp


def kernel(t, x, mask, W_embed, b_embed, W0, b0, W1, b1, W2, b2, W3, b3):
    global last_results, _C60_H
    from concourse import bass_utils

    mask = np.asarray(mask)
    B, Amax = mask.shape
    N = B * Amax

    in_maps, pad, nact, h, C60, xp = _prep_host(
        t, x, mask, W_embed, b_embed, W0, b0, W1, b1, W2, b2, W3, b3)
    _C60_H = (C60, h)

    nc = build_bass()
    res = bass_utils.run_bass_kernel_spmd(nc, in_maps,
                                          core_ids=list(range(N_CORES)))
    last_results = res

    zall = np.concatenate([r["zout"].T for r in res.results], 0)  # (total, E)
    out = np.zeros((N, E), np.float32)
    out[pad[:nact]] = zall[:nact]
    return out.reshape(B, Amax, E)


# revision 42
# speedup vs baseline: 1.2154x; 1.2154x over previous
"""Trainium2 Bass kernel v4 for nn_CDE — feature-major, bf16, all-DVE einsum.

Structure (per core, Nc=136 lanes, 60 RK stages):
- Host precomputes: natural-spline derivative rows dx[s,n,d] (C60 @ knots)
  with the RK stage scale c_r folded in, pre-broadcast to 128 partitions in
  DRAM (bf16); the embed z0; weight transposes in bf16.
- Device stage: feature-major MLP (L0 4mm, L1/L2 16mm, L3 40mm, all bf16),
  tanh (bf16 out) with native per-partition bias on ACT.  The einsum is pure
  DVE: per d-pair a dense bf16 multiply against the DMA'd dx slice (2x mode,
  ~200ns) and an f32 running add; one fold -> kbf.  Post-tanh tail ~0.75us
  keeps the PE HAM clock warm (2.4 GHz).  RK updates (SBUF-only, ~1 stage of
  slack) run on gpsimd to keep DVE free.
"""
import os
import sys
import types

for _p in ("/opt/trn_rl_repo", "/root/.axon_site/_ro/trn_rl_repo"):
    if os.path.isdir(_p) and _p not in sys.path:
        sys.path.insert(0, _p)

if "antenv.axon_hooks" not in sys.modules:
    _m = types.ModuleType("antenv.axon_hooks")
    _hook = [None]

    def _set(hook):
        _hook[0] = hook

    def _get():
        if _hook[0] is None:
            try:
                from trn_agent_boot.trn_boot import _ntff_profile_via_ctypes
                _hook[0] = _ntff_profile_via_ctypes("/opt/axon/libaxon_pjrt.so")
            except Exception:
                pass
        return _hook[0]

    _m.set_axon_ntff_profile_hook = _set
    _m.get_axon_ntff_profile_hook = _get
    sys.modules["antenv.axon_hooks"] = _m

import numpy as np

N_CORES = 8
T, D, E, H = 16, 10, 128, 512
F3 = E * D
N_STEPS = T - 1
N_STAGES = 4 * N_STEPS  # 60
NC = 136               # lanes per core (8*136 = 1088 >= nact)
DNC = D * NC

last_results = None


def spline_stage_matrix(t):
    t = np.asarray(t, np.float64)
    Tn = len(t)
    h = np.diff(t)
    A = np.zeros((Tn, Tn))
    A[0, 0] = 1.0
    A[-1, -1] = 1.0
    for i in range(1, Tn - 1):
        A[i, i - 1] = h[i - 1]
        A[i, i] = 2.0 * (h[i - 1] + h[i])
        A[i, i + 1] = h[i]
    R = np.zeros((Tn, Tn))
    for i in range(1, Tn - 1):
        R[i, i - 1] = 6.0 / h[i - 1]
        R[i, i] = -6.0 / h[i - 1] - 6.0 / h[i]
        R[i, i + 1] = 6.0 / h[i]
    S = np.linalg.solve(A, R)
    Iden = np.eye(Tn)
    rows = []
    for j in range(Tn - 1):
        hs = h[j]
        for u_frac in (0.0, 1.0 / 3.0, 2.0 / 3.0, 1.0):
            s = t[j + 1] if u_frac == 1.0 else t[j] + u_frac * hs
            i = int(np.clip(np.searchsorted(t, s, side="right") - 1, 0, Tn - 2))
            u = s - t[i]
            b_row = (Iden[i + 1] - Iden[i]) / h[i] - h[i] * (2.0 * S[i] + S[i + 1]) / 6.0
            rows.append(b_row + u * S[i] + (u * u) / (2.0 * h[i]) * (S[i + 1] - S[i]))
    return np.asarray(rows), h


def w3_perm():
    fp = np.arange(F3)
    return (fp % E) * D + fp // E


def rk_scales(h):
    """c_r per stage so k'_r = c_r * k_r makes all RK updates plain adds."""
    c = np.empty(N_STAGES)
    for j in range(N_STEPS):
        hs = h[j]
        c[4 * j + 0] = hs / 3.0
        c[4 * j + 1] = hs
        c[4 * j + 2] = hs
        c[4 * j + 3] = hs / 8.0
    return c


def build_bass():
    import concourse.bass as bass  # noqa: F401
    import concourse.bacc as bacc
    import concourse.tile as tile
    import concourse.mybir as mybir

    F32 = mybir.dt.float32
    BF16 = mybir.dt.bfloat16
    AF = mybir.ActivationFunctionType
    ALU = mybir.AluOpType

    nc = bacc.Bacc("TRN2", target_bir_lowering=False)

    d_zin0 = nc.dram_tensor("zin0", [E, NC], BF16, kind="ExternalInput")
    d_z0 = nc.dram_tensor("z0", [E, NC], F32, kind="ExternalInput")
    d_dxall = nc.dram_tensor("dxall", [128, N_STAGES * DNC], BF16,
                             kind="ExternalInput")
    d_w0 = nc.dram_tensor("w0t", [E, H], BF16, kind="ExternalInput")
    d_w1 = nc.dram_tensor("w1t", [H, H], BF16, kind="ExternalInput")
    d_w2 = nc.dram_tensor("w2t", [H, H], BF16, kind="ExternalInput")
    d_w3 = nc.dram_tensor("w3pt", [H, F3], BF16, kind="ExternalInput")
    d_b012 = nc.dram_tensor("b012", [E, 12], F32, kind="ExternalInput")
    d_b3t = nc.dram_tensor("b3t", [2, 5 * E], BF16, kind="ExternalInput")
    d_oh2 = nc.dram_tensor("oh2", [2, 2 * NC], BF16, kind="ExternalInput")
    d_out = nc.dram_tensor("zout", [E, NC], F32, kind="ExternalOutput")

    with tile.TileContext(nc) as tc:
        with (
            tc.tile_pool(name="wpool", bufs=1) as wpool,
            tc.tile_pool(name="apool", bufs=2) as apool,
            tc.tile_pool(name="dpool", bufs=3) as dpool,
            tc.tile_pool(name="pmlp", bufs=3, space="PSUM") as pmlp,
            tc.tile_pool(name="p3p", bufs=4, space="PSUM") as p3p,
            tc.tile_pool(name="pjunk", bufs=1, space="PSUM") as pjunk,
        ):
            w0t = wpool.tile([E, H], BF16, tag="w0t")
            nc.sync.dma_start(out=w0t, in_=d_w0[:, :])
            w1k = [wpool.tile([128, H], BF16, tag=f"w1k{k}", name=f"w1k{k}")
                   for k in range(4)]
            w2k = [wpool.tile([128, H], BF16, tag=f"w2k{k}", name=f"w2k{k}")
                   for k in range(4)]
            w3k = [wpool.tile([128, F3], BF16, tag=f"w3k{k}", name=f"w3k{k}")
                   for k in range(4)]
            for k in range(4):
                nc.sync.dma_start(out=w1k[k], in_=d_w1[128 * k:128 * (k + 1), :])
                nc.sync.dma_start(out=w2k[k], in_=d_w2[128 * k:128 * (k + 1), :])
                nc.sync.dma_start(out=w3k[k], in_=d_w3[128 * k:128 * (k + 1), :])
            b012 = wpool.tile([E, 12], F32, tag="b012")
            nc.sync.dma_start(out=b012, in_=d_b012[:, :])
            b3t = wpool.tile([2, 5 * E], BF16, tag="b3t")
            nc.sync.dma_start(out=b3t, in_=d_b3t[:, :])
            onehot2 = wpool.tile([2, 2 * NC], BF16, tag="oh2")
            nc.sync.dma_start(out=onehot2, in_=d_oh2[:, :])

            z0t = wpool.tile([E, NC], F32, tag="z0in")
            nc.sync.dma_start(out=z0t, in_=d_z0[:, :])
            zin0 = wpool.tile([E, NC], BF16, tag="zin0")
            nc.sync.dma_start(out=zin0, in_=d_zin0[:, :])

            junk = pjunk.tile([128, 512], F32, tag="junk")

            def fillers(n, rhs_ap):
                # keep-warm matmuls into a scratch PSUM bank: the PE HAM
                # clock gate re-throttles to 1.2 GHz if the PE sees idle
                # windows, so plug dependency-wait gaps with junk work.
                # rhs_ap gates WHEN they become runnable (the scheduler
                # reorders the PE stream, so data deps are the only
                # reliable placement).
                rhs_ap, fs = rhs_ap
                for i in range(n):
                    nc.tensor.matmul(junk[:, 0:fs], w0t[:, 0:128],
                                     rhs_ap, start=True, stop=True)

            dxb = {}

            def fetch_dx(s):
                if s >= N_STAGES:
                    return
                tl = dpool.tile([128, DNC], BF16, tag="dxb", name=f"dxb_{s}")
                nc.sync.dma_start(out=tl, in_=d_dxall[:, s * DNC:(s + 1) * DNC])
                dxb[s] = tl

            fetch_dx(0)
            fetch_dx(1)

            kp = [None] * 4
            z = z0t
            zjbf = zin0
            zb3bf = zb4bf = zprebf = zpre = s12 = None
            p0_pend = None  # psum pair tiles with base already accumulated
            kbf = None

            def relu(eng, out_ap, in_ap, bias_ap):
                if eng == "dve":
                    nc.vector.tensor_scalar(out=out_ap, in0=in_ap,
                                            scalar1=bias_ap, scalar2=0.0,
                                            op0=ALU.add, op1=ALU.max)
                else:
                    nc.scalar.activation(out_ap, in_ap, AF.Relu,
                                         bias=bias_ap, scale=1.0)

            def TT(out_ap, a_ap, b_ap, op=ALU.add):
                nc.vector.tensor_tensor(out=out_ap, in0=a_ap, in1=b_ap, op=op)

            def GT(out_ap, a_ap, b_ap, op=ALU.add):
                nc.vector.tensor_tensor(out=out_ap, in0=a_ap, in1=b_ap, op=op)

            R_ENG = ["dve", "act", "dve", "act"]

            for s in range(N_STAGES):
                j, r = divmod(s, 4)
                last = s == N_STAGES - 1

                # ---- L0 (psum may already hold base accumulation)
                if p0_pend is None:
                    p0 = [pmlp.tile([128, 2, 256], F32, tag="pmlp",
                                    name=f"p0a_{s}"),
                          pmlp.tile([128, 2, 256], F32, tag="pmlp",
                                    name=f"p0b_{s}")]
                    for m in range(4):
                        nc.tensor.matmul(p0[m >> 1][:, m & 1, 0:NC],
                                         w0t[:, 128 * m:128 * (m + 1)],
                                         zin0[:, :], start=True, stop=True)
                else:
                    p0 = p0_pend
                    for m in range(4):
                        nc.tensor.matmul(p0[m >> 1][:, m & 1, 0:NC],
                                         w0t[:, 128 * m:128 * (m + 1)],
                                         kbf[:, :],
                                         start=False, stop=((m & 1) == 1))
                fillers(4, (kbf[:, :], NC) if kbf is not None
                        else (zin0[:, :], NC))
                y0 = apool.tile([128, 4, NC], BF16, tag="y0", name=f"y0_{s}")
                for m in range(4):
                    relu(R_ENG[m], y0[:, m, :], p0[m >> 1][:, m & 1, 0:NC],
                         b012[:, m:m + 1])
                fetch_dx(s + 2)
                # ---- L1 (k-major so matmuls start after first relu chunk)
                p1 = [pmlp.tile([128, 2, 256], F32, tag="pmlp", name=f"p1a_{s}"),
                      pmlp.tile([128, 2, 256], F32, tag="pmlp", name=f"p1b_{s}")]
                for m in range(4):
                    for k in range(4):
                        nc.tensor.matmul(p1[m >> 1][:, m & 1, 0:NC],
                                         w1k[k][:, 128 * m:128 * (m + 1)],
                                         y0[:, k, :],
                                         start=((m & 1) == 0 and k == 0),
                                         stop=((m & 1) == 1 and k == 3))
                fillers(4, (y0[:, 0, :], NC))
                y1 = apool.tile([128, 4, NC], BF16, tag="y1", name=f"y1_{s}")
                for m in range(4):
                    relu(R_ENG[m], y1[:, m, :], p1[m >> 1][:, m & 1, 0:NC],
                         b012[:, 4 + m:5 + m])
                # ---- L2
                p2 = [pmlp.tile([128, 2, 256], F32, tag="pmlp", name=f"p2a_{s}"),
                      pmlp.tile([128, 2, 256], F32, tag="pmlp", name=f"p2b_{s}")]
                for m in range(4):
                    for k in range(4):
                        nc.tensor.matmul(p2[m >> 1][:, m & 1, 0:NC],
                                         w2k[k][:, 128 * m:128 * (m + 1)],
                                         y1[:, k, :],
                                         start=((m & 1) == 0 and k == 0),
                                         stop=((m & 1) == 1 and k == 3))
                fillers(4, (y1[:, 0, :], NC))
                y2 = apool.tile([128, 4, NC], BF16, tag="y2", name=f"y2_{s}")
                for m in range(4):
                    relu(R_ENG[m], y2[:, m, :], p2[m >> 1][:, m & 1, 0:NC],
                         b012[:, 8 + m:9 + m])

                # ---- L3 + tanh; bias is pre-added into PSUM by a K=2
                # one-hot matmul so each d-pair is ONE [128,272] tanh.  The
                # einsum is a per-pair dense bf16 multiply + bf16 running
                # add, so only pair 4's mult/add trail the last tanh.
                dxs = dxb.pop(s)
                sacc = None
                for p in range(5):
                    p3 = p3p.tile([128, 2, 256], F32, tag="p3", name=f"p3_{s}_{p}")
                    p3v = bass.AP(tensor=p3.tensor, offset=p3.offset,
                                  ap=[p3.ap[0], [256, 2], [1, NC]])
                    nc.tensor.matmul(p3v, b3t[:, 128 * p:128 * (p + 1)],
                                     onehot2[:, :], start=True, stop=True)
                    for half in range(2):
                        dd = 2 * p + half
                        for k in range(4):
                            nc.tensor.matmul(p3[:, half, 0:NC],
                                             w3k[k][:, 128 * dd:128 * (dd + 1)],
                                             y2[:, k, :], start=False,
                                             stop=(k == 3))
                    y3 = apool.tile([128, 2 * NC], BF16, tag="y3",
                                    name=f"y3_{s}_{p}")
                    nc.scalar.activation(y3, p3v, AF.Tanh, bias=0.0, scale=1.0)
                    tmp = apool.tile([128, 2 * NC], BF16, tag="tmp",
                                     name=f"tmp_{s}_{p}")
                    TT(tmp, y3, dxs[:, 2 * p * NC:(2 * p + 2) * NC],
                       op=ALU.mult)
                    if p == 0:
                        sacc = tmp
                    else:
                        a = apool.tile([128, 2 * NC], BF16, tag="sacc",
                                       name=f"sacc_{s}_{p}")
                        TT(a, sacc, tmp)
                        sacc = a
                    if p == 3:
                        fillers(3, (y3[:, 0:NC], NC))
                    elif p == 4:
                        fillers(8, (y3[:, 0:NC], NC))

                # ---- off-path RK partials (gpsimd, SBUF-only, ~1 stage slack)
                if r == 0 and j > 0:
                    znew = apool.tile([E, NC], F32, tag="z", name=f"z_{j}")
                    GT(znew, zpre, kp[3])
                    z = znew
                    zjbf = apool.tile([E, NC], BF16, tag="zjbf", name=f"zjbf_{j}")
                    GT(zjbf, zpre, kp[3])
                elif r == 1:
                    zb3bf = apool.tile([E, NC], BF16, tag="zb3", name=f"zb3_{j}")
                    GT(zb3bf, z, kp[0], op=ALU.subtract)
                elif r == 2:
                    t4 = apool.tile([E, NC], F32, tag="t4", name=f"t4_{j}")
                    nc.vector.scalar_tensor_tensor(
                        out=t4, in0=kp[0], scalar=3.0, in1=z,
                        op0=ALU.mult, op1=ALU.add)
                    zb4bf = apool.tile([E, NC], BF16, tag="zb4", name=f"zb4_{j}")
                    GT(zb4bf, t4, kp[1], op=ALU.subtract)
                    s12 = apool.tile([E, NC], F32, tag="s12", name=f"s12_{j}")
                    GT(s12, kp[0], kp[1])
                elif r == 3:
                    s123 = apool.tile([E, NC], F32, tag="s123", name=f"s123_{j}")
                    GT(s123, s12, kp[2])
                    zpre = apool.tile([E, NC], F32, tag="zpre", name=f"zpre_{j}")
                    nc.vector.scalar_tensor_tensor(
                        out=zpre, in0=s123, scalar=0.375, in1=z,
                        op0=ALU.mult, op1=ALU.add)
                    zprebf = apool.tile([E, NC], BF16, tag="zprebf",
                                        name=f"zprebf_{j}")
                    nc.vector.tensor_copy(out=zprebf, in_=zpre)

                # ---- base L0 for next stage (runs during this stage's tail)
                if not last:
                    rn = (r + 1) % 4
                    base = (zprebf, zjbf, zb3bf, zb4bf)[rn]
                    p0_pend = [pmlp.tile([128, 2, 256], F32, tag="pmlp",
                                         name=f"p0a_{s + 1}"),
                               pmlp.tile([128, 2, 256], F32, tag="pmlp",
                                         name=f"p0b_{s + 1}")]
                    for m in range(4):
                        nc.tensor.matmul(p0_pend[m >> 1][:, m & 1, 0:NC],
                                         w0t[:, 128 * m:128 * (m + 1)],
                                         base[:, :],
                                         start=((m & 1) == 0), stop=False)

                # ---- finish k' (bf16, feeds both the L0 accumulation and
                # the RK updates)
                kbf = apool.tile([E, NC], BF16, tag="kbf", name=f"kbf_{s}")
                TT(kbf, sacc[:, 0:NC], sacc[:, NC:2 * NC])
                kp[r] = kbf

                if last:
                    zfin = apool.tile([E, NC], F32, tag="zfin", name="zfin")
                    TT(zfin, zpre, kbf)
                    z = zfin

            nc.sync.dma_start(out=d_out[:, :], in_=z)
    nc.finalize()
    return nc


_C60_H = None


def _prep_host(t, x, mask, W_embed, b_embed, W0, b0, W1, b1, W2, b2, W3, b3):
    import ml_dtypes
    bf16 = ml_dtypes.bfloat16

    t = np.asarray(t, np.float32)
    x = np.asarray(x, np.float32)
    mask = np.asarray(mask)
    B, Amax = mask.shape
    N = B * Amax

    C60, h = spline_stage_matrix(t)
    C60 = C60.astype(np.float32)
    idx = np.flatnonzero(mask.ravel())
    nact = len(idx)
    total = N_CORES * NC
    assert nact <= total, f"nact={nact} > {total}"
    pad = np.full(total, idx[0] if nact else 0, dtype=np.int64)
    pad[:nact] = idx
    xp = x.reshape(N, T, D)[pad]  # (total, T, D)

    c = rk_scales(h).astype(np.float32)
    perm = w3_perm()
    W3p = np.asarray(W3, np.float32)[perm]
    b3pv = np.asarray(b3, np.float32)[perm]

    shared = dict(
        w0t=np.ascontiguousarray(np.asarray(W0).T).astype(bf16),
        w1t=np.ascontiguousarray(np.asarray(W1).T).astype(bf16),
        w2t=np.ascontiguousarray(np.asarray(W2).T).astype(bf16),
        w3pt=np.ascontiguousarray(W3p.T).astype(bf16),
        b012=np.stack([np.asarray(b, np.float32)[m * 128:(m + 1) * 128]
                       for b in (b0, b1, b2) for m in range(4)],
                      axis=1).astype(np.float32),
        b3t=np.ascontiguousarray(
            b3pv.reshape(5, 2, E).transpose(1, 0, 2).reshape(2, 5 * E)
        ).astype(bf16),
        oh2=np.kron(np.eye(2, dtype=np.float32),
                    np.ones((1, NC), np.float32)).astype(bf16),
    )

    Wemb = np.asarray(W_embed, np.float32)
    bemb = np.asarray(b_embed, np.float32)
    in_maps = []
    for core in range(N_CORES):
        xc = xp[core * NC:(core + 1) * NC]  # (NC, T, D)
        dx = np.einsum("st,ntd->snd", C60, xc)  # (60, NC, D)
        # (60, D, NC) row per stage, scaled by c; broadcast to 128 partitions
        dxc = (dx.transpose(0, 2, 1) * c[:, None, None]).reshape(1, N_STAGES * DNC)
        dxall = np.ascontiguousarray(
            np.broadcast_to(dxc, (128, N_STAGES * DNC))).astype(bf16)
        z0 = (xc[:, 0, :] @ Wemb.T + bemb).astype(np.float32).T  # (E, NC)
        in_maps.append(dict(
            zin0=np.ascontiguousarray(z0).astype(bf16),
            z0=np.ascontiguousarray(z0),
            dxall=dxall,
            **shared,
        ))
    return in_maps, pad, nact, h, C60, xp


def kernel(t, x, mask, W_embed, b_embed, W0, b0, W1, b1, W2, b2, W3, b3):
    global last_results, _C60_H
    from concourse import bass_utils

    mask = np.asarray(mask)
    B, Amax = mask.shape
    N = B * Amax

    in_maps, pad, nact, h, C60, xp = _prep_host(
        t, x, mask, W_embed, b_embed, W0, b0, W1, b1, W2, b2, W3, b3)
    _C60_H = (C60, h)

    nc = build_bass()
    res = bass_utils.run_bass_kernel_spmd(nc, in_maps,
                                          core_ids=list(range(N_CORES)))
    last_results = res

    zall = np.concatenate([r["zout"].T for r in res.results], 0)  # (total, E)
    out = np.zeros((N, E), np.float32)
    out[pad[:nact]] = zall[:nact]
    return out.reshape(B, Amax, E)
